# revision 4
# baseline (speedup 1.0000x reference)
"""Segment-average pooling kernel for Trainium2 (8 NeuronCores, SPMD).

Algorithm (per batch, data-parallel over batches across cores):
  The reference computes, for each token t, the mean of encoded_feats rows
  f in [s_t, e_t] (inclusive), where s/e are truncated fractional
  alignments. Instead of the O(Tt*Tf*D) masked einsum, we compute an
  exclusive prefix-sum table over frames and reduce each segment as a
  difference of two table rows:

    P_excl[g] = sum_{f < g} X[f]        (g in 0..Tf)
    seg_sum_t = P_excl[e_t + 1] - P_excl[s_t]

  The prefix table is built in two levels so the PE does all heavy work:
    - T[blk*128 + r] = in-block exclusive cumsum (strict-upper-tri matmul)
    - R[blk]         = sum of full blocks before blk (tiny matmul)
    so P_excl[g] = T[g] + R[g >> 7].
  T (4097 x 512 f32) is written to DRAM; one indirect DMA gathers the 2048
  rows needed (end/start per token). The R part is applied post-gather via
  a small matmul against host-built {-1,0,+1} one-hot matrices. Final
  (end - start + corr) * (1/count) runs on DVE/ACT.

  All index arithmetic (s/e truncation, counts, one-hots) is O(B*Tt)
  metadata computed on host, exactly mirroring reference semantics.
"""

import numpy as np
from contextlib import ExitStack

import concourse.bacc as bacc
import concourse.bass as bass
import concourse.tile as tile
from concourse import mybir
from concourse.bass import IndirectOffsetOnAxis
from concourse.bass_utils import run_bass_kernel_spmd

B, TF, D, TT = 16, 4096, 512, 1024
NCORES = 8
BPC = B // NCORES          # batches per core
NBLK = TF // 128           # 32 frame blocks
GRP = 4                    # blocks per DMA group
NJ = TT // 128             # 8 tokens per partition
F32 = mybir.dt.float32

_cache: dict = {}


def build_bass():
    nc = bacc.Bacc(
        "TRN2", target_bir_lowering=False, debug=False, num_devices=NCORES
    )
    feats = nc.dram_tensor("feats", [BPC, TF, D], F32, kind="ExternalInput")
    gidx = nc.dram_tensor(
        "gidx", [BPC, 128, 2 * NJ], mybir.dt.int32, kind="ExternalInput"
    )
    recip = nc.dram_tensor("recip", [BPC, 128, NJ], F32, kind="ExternalInput")
    ohd = nc.dram_tensor(
        "ohd", [BPC, NBLK + 1, NJ, 128], F32, kind="ExternalInput"
    )
    tri = nc.dram_tensor("tri", [128, 128], F32, kind="ExternalInput")
    sel = nc.dram_tensor("sel", [128, 63], F32, kind="ExternalInput")
    rtri = nc.dram_tensor("rtri", [NBLK, NBLK + 1], F32, kind="ExternalInput")
    out = nc.dram_tensor("out", [BPC, TT, D], F32, kind="ExternalOutput")
    ttabs = [
        nc.dram_tensor(f"ttab{b}", [TF + 1, D], F32) for b in range(BPC)
    ]

    with tile.TileContext(nc) as tc, ExitStack() as ctx:
        consts = ctx.enter_context(tc.tile_pool(name="consts", bufs=1))
        xp = ctx.enter_context(tc.tile_pool(name="xp", bufs=3))
        tp = ctx.enter_context(tc.tile_pool(name="tp", bufs=3))
        srp = ctx.enter_context(tc.tile_pool(name="srp", bufs=2))
        small = ctx.enter_context(tc.tile_pool(name="small", bufs=2))
        gathp = ctx.enter_context(tc.tile_pool(name="gathp", bufs=2))
        outp = ctx.enter_context(tc.tile_pool(name="outp", bufs=2))
        tmpp = ctx.enter_context(tc.tile_pool(name="tmpp", bufs=3))
        cps = ctx.enter_context(tc.tile_pool(name="cps", bufs=3, space="PSUM"))
        sps = ctx.enter_context(tc.tile_pool(name="sps", bufs=1, space="PSUM"))
        rps = ctx.enter_context(tc.tile_pool(name="rps", bufs=1, space="PSUM"))
        corrps = ctx.enter_context(
            tc.tile_pool(name="corrps", bufs=2, space="PSUM")
        )

        tri_sb = consts.tile([128, 128], F32)
        nc.sync.dma_start(tri_sb[:], tri.ap())
        sel_sb = consts.tile([128, 63], F32)
        nc.sync.dma_start(sel_sb[:], sel.ap())
        rtri_sb = consts.tile([NBLK, NBLK + 1], F32)
        nc.sync.dma_start(rtri_sb[:], rtri.ap())
        zrow = consts.tile([1, D], F32)
        nc.vector.memset(zrow[:], 0.0)

        for b in range(BPC):
            xview = feats.ap()[b].rearrange("(n p) d -> p n d", p=128)
            tview = ttabs[b].ap()[0:TF, :].rearrange("(n p) d -> p n d", p=128)
            nc.sync.dma_start(ttabs[b].ap()[TF : TF + 1, :], zrow[:])

            idx_sb = small.tile([128, 2 * NJ], mybir.dt.int32)
            nc.sync.dma_start(idx_sb[:], gidx.ap()[b])
            rec_sb = small.tile([128, NJ], F32)
            nc.sync.dma_start(rec_sb[:], recip.ap()[b])
            ohd_sb = small.tile([NBLK + 1, NJ, 128], F32)
            nc.sync.dma_start(ohd_sb[:], ohd.ap()[b])

            s_ps = sps.tile([NBLK, D], F32)
            for g in range(NBLK // GRP):
                x4 = xp.tile([128, GRP, D], F32)
                nc.sync.dma_start(x4[:], xview[:, g * GRP : (g + 1) * GRP, :])
                t4 = tp.tile([128, GRP, D], F32)
                for i in range(GRP):
                    blk = g * GRP + i
                    nc.tensor.matmul(
                        s_ps[:],
                        lhsT=sel_sb[:, 31 - blk : 63 - blk],
                        rhs=x4[:, i, :],
                        start=(blk == 0),
                        stop=(blk == NBLK - 1),
                    )
                for i in range(GRP):
                    c_ps = cps.tile([128, D], F32)
                    nc.tensor.matmul(
                        c_ps[:], lhsT=tri_sb[:], rhs=x4[:, i, :],
                        start=True, stop=True,
                    )
                    if i % 2 == 0:
                        nc.scalar.copy(t4[:, i, :], c_ps[:])
                    else:
                        nc.vector.tensor_copy(t4[:, i, :], c_ps[:])
                nc.sync.dma_start(tview[:, g * GRP : (g + 1) * GRP, :], t4[:])

            s_sb = srp.tile([NBLK, D], F32)
            nc.vector.tensor_copy(s_sb[:], s_ps[:])
            r_ps = rps.tile([NBLK + 1, D], F32)
            nc.tensor.matmul(
                r_ps[:], lhsT=rtri_sb[:], rhs=s_sb[:], start=True, stop=True
            )
            r_sb = srp.tile([NBLK + 1, D], F32)
            nc.vector.tensor_copy(r_sb[:], r_ps[:])

            gath = gathp.tile([128, 2 * NJ, D], F32)
            for j in range(2 * NJ):
                nc.gpsimd.indirect_dma_start(
                    out=gath[:, j, :],
                    out_offset=None,
                    in_=ttabs[b].ap(),
                    in_offset=IndirectOffsetOnAxis(
                        ap=idx_sb[:, j : j + 1], axis=0
                    ),
                )

            outsb = outp.tile([128, NJ, D], F32)
            for j in range(NJ):
                corr = corrps.tile([128, D], F32)
                nc.tensor.matmul(
                    corr[:], lhsT=ohd_sb[:, j, :], rhs=r_sb[:],
                    start=True, stop=True,
                )
                dif = tmpp.tile([128, D], F32)
                nc.vector.tensor_tensor(
                    out=dif[:], in0=gath[:, j, :], in1=gath[:, NJ + j, :],
                    op=mybir.AluOpType.subtract,
                )
                nc.vector.tensor_add(out=dif[:], in0=dif[:], in1=corr[:])
                nc.scalar.mul(outsb[:, j, :], dif[:], rec_sb[:, j : j + 1])
            nc.sync.dma_start(
                out.ap()[b].rearrange("(p j) d -> p j d", p=128), outsb[:]
            )

    nc.compile()
    return nc


def _host_metadata(encoded_feat_lengths, asr_token_lengths, asr_token_alignments):
    """Index metadata, exactly mirroring reference truncation semantics."""
    L = encoded_feat_lengths.astype(np.float32)                      # [B]
    prod = asr_token_alignments.astype(np.float32) * L[:, None, None]
    se = prod.astype(np.int32)                                       # trunc
    s_, e_ = se[..., 0], se[..., 1]
    valid = np.arange(TT, dtype=np.int32)[None, :] < asr_token_lengths[:, None]
    e_c = np.minimum(e_, TF - 1)
    cnt = np.where(valid, np.maximum(e_c - s_ + 1, 0), 0)
    ok = cnt > 0
    g_e = np.where(ok, e_c + 1, 0).astype(np.int32)
    g_s = np.where(ok, s_, 0).astype(np.int32)
    rec = np.where(
        ok, np.float32(1.0) / np.maximum(cnt, 1).astype(np.float32), np.float32(0.0)
    ).astype(np.float32)

    gidx = np.concatenate(
        [g_e.reshape(B, 128, NJ), g_s.reshape(B, 128, NJ)], axis=2
    ).astype(np.int32)
    recip = rec.reshape(B, 128, NJ)

    be = (g_e >> 7).astype(np.int64)
    bs = (g_s >> 7).astype(np.int64)
    ohd = np.zeros((B, NBLK + 1, NJ, 128), np.float32)
    t_idx = np.arange(TT, dtype=np.int64)
    p_idx = t_idx // NJ
    j_idx = t_idx % NJ
    b_idx = np.repeat(np.arange(B, dtype=np.int64), TT)
    okf = ok.ravel()
    bef = be.ravel()
    bsf = bs.ravel()
    pf = np.tile(p_idx, B)
    jf = np.tile(j_idx, B)
    np.add.at(ohd, (b_idx[okf], bef[okf], jf[okf], pf[okf]), np.float32(1.0))
    np.add.at(ohd, (b_idx[okf], bsf[okf], jf[okf], pf[okf]), np.float32(-1.0))
    return gidx, recip, ohd


def _host_constants():
    tri = np.triu(np.ones((128, 128), np.float32), k=1)
    sel = np.zeros((128, 63), np.float32)
    sel[:, 31] = 1.0
    rtri = np.triu(np.ones((NBLK, NBLK + 1), np.float32), k=1)
    return tri, sel, rtri


def kernel(
    encoded_feats,
    encoded_feat_lengths,
    asr_token_ids,
    asr_token_lengths,
    asr_token_alignments,
):
    encoded_feats = np.asarray(encoded_feats, dtype=np.float32)
    encoded_feat_lengths = np.asarray(encoded_feat_lengths, dtype=np.int32)
    asr_token_lengths = np.asarray(asr_token_lengths, dtype=np.int32)
    asr_token_alignments = np.asarray(asr_token_alignments, dtype=np.float32)

    if "nc" not in _cache:
        _cache["nc"] = build_bass()
    nc = _cache["nc"]

    gidx, recip, ohd = _host_metadata(
        encoded_feat_lengths, asr_token_lengths, asr_token_alignments
    )
    tri, sel, rtri = _host_constants()

    in_maps = []
    for c in range(NCORES):
        sl = slice(c * BPC, (c + 1) * BPC)
        in_maps.append(
            {
                "feats": np.ascontiguousarray(encoded_feats[sl]),
                "gidx": np.ascontiguousarray(gidx[sl]),
                "recip": np.ascontiguousarray(recip[sl]),
                "ohd": np.ascontiguousarray(ohd[sl]),
                "tri": tri,
                "sel": sel,
                "rtri": rtri,
            }
        )

    res = run_bass_kernel_spmd(nc, in_maps, core_ids=list(range(NCORES)))
    seg = np.concatenate(
        [res.results[c]["out"] for c in range(NCORES)], axis=0
    ).astype(np.float32)
    return seg, asr_token_lengths
